# revision 1
# baseline (speedup 1.0000x reference)
"""Trainium2 Bass kernel for causal multi-head attention with RoPE.

Problem: x[2,2048,2048] -> qkv proj -> RoPE(q,k) -> causal softmax attention
(16 heads, hd=128) -> out proj.  Sharding: tensor-parallel over heads
(2 heads/core x 8 cores); the output projection contraction is restored
with one AllToAll per batch (head-shards -> sequence-shards), overlapped
with the other batch's compute, so each core computes a disjoint
[2, 256, 2048] slice of the final output.

All matmuls run as float32r (full-rate fp32 PE mode, ~1.6e-4 rel err on a
2048-deep contraction).  Softmax skips the max-subtraction (scores are
O(1) by construction); the causal mask is accumulated into PSUM as a
-1e9 constant via a PE identity-matmul; softmax denominators are
partition-reduced and broadcast back with tiny ones-matmuls on the PE.
"""

import os
import sys

if "/opt/trn_rl_repo" not in sys.path:
    sys.path.insert(0, "/opt/trn_rl_repo")

import numpy as np

B, S, D = 2, 2048, 2048
H, HD = 16, 128
NCORES = 8
HPC = H // NCORES          # heads per core (2)
ROPE_BASE = 10000.0
SCALE = 1.0 / float(np.sqrt(HD))
SC = 512                   # QKV matmul free-dim chunk (s positions)
KSUB = D // 128            # 16 contraction subtiles
SCW = S // NCORES          # 256: per-core output cols per batch

_CACHE = {}


def _install_trace_shim():
    """Optionally register the axon NTFF profile hook (for test.py tracing)."""
    try:
        import types

        if "antenv.axon_hooks" in sys.modules:
            return True
        import antenv
        from trn_agent_boot.trn_boot import _ntff_profile_via_ctypes

        hook = _ntff_profile_via_ctypes("/opt/axon/libaxon_pjrt.so")
        mod = types.ModuleType("antenv.axon_hooks")
        _state = {"hook": hook}
        mod.get_axon_ntff_profile_hook = lambda: _state["hook"]
        mod.set_axon_ntff_profile_hook = lambda h: _state.__setitem__("hook", h)
        sys.modules["antenv.axon_hooks"] = mod
        antenv.axon_hooks = mod
        return True
    except Exception:
        return False


def _build():
    import concourse.bass as bass  # noqa: F401
    import concourse.mybir as mybir
    import concourse.tile as tile
    from concourse import bacc
    from concourse.masks import make_identity

    f32 = mybir.dt.float32
    f32r = mybir.dt.float32r
    EXP = mybir.ActivationFunctionType.Exp

    nc = bacc.Bacc("TRN2", target_bir_lowering=False, debug=False,
                   num_devices=NCORES)

    xT = nc.dram_tensor("xT", [128, KSUB, B * S], f32r, kind="ExternalInput")
    wqkv = nc.dram_tensor("wqkv", [128, KSUB, 3 * HPC * HD], f32r,
                          kind="ExternalInput")
    wout = nc.dram_tensor("wout", [128, KSUB, D], f32r, kind="ExternalInput")
    cosg = nc.dram_tensor("cosg", [128, S], f32, kind="ExternalInput")
    sing = nc.dram_tensor("sing", [128, S], f32, kind="ExternalInput")
    mneg = nc.dram_tensor("mneg", [128, 512], f32r, kind="ExternalInput")
    y = nc.dram_tensor("y", [B, SCW, D], f32, kind="ExternalOutput")

    NQC = S // SC          # qkv s-chunks per batch
    NKT = S // 128         # 16 key tiles
    VOFF = 2 * HPC * HD    # v block column offset in w_sb (512)

    with tile.TileContext(nc) as tc:
        with tc.tile_pool(name="const", bufs=1) as cp, \
             tc.tile_pool(name="stage", bufs=1) as stp, \
             tc.tile_pool(name="dram", bufs=1, space="DRAM") as dp, \
             tc.tile_pool(name="psA", bufs=4, space="PSUM") as psA, \
             tc.tile_pool(name="psOut", bufs=1, space="PSUM") as psO, \
             tc.tile_pool(name="w", bufs=1) as wp, \
             tc.tile_pool(name="xc", bufs=2) as xp, \
             tc.tile_pool(name="qkv", bufs=1) as qp, \
             tc.tile_pool(name="attn", bufs=1) as ap_, \
             tc.tile_pool(name="rotp", bufs=1) as rp, \
             tc.tile_pool(name="small", bufs=4) as ep:

            cos_sb = cp.tile([128, S], f32, name="cos_sb")
            sin_sb = cp.tile([128, S], f32, name="sin_sb")
            mneg_sb = cp.tile([128, 512], f32r, name="mneg_sb")
            ident = cp.tile([128, 128], f32, name="ident")
            identR = cp.tile([128, 128], f32r, name="identR")
            onesc = cp.tile([128, 1], f32, name="onesc")
            onescR = cp.tile([128, 1], f32r, name="onescR")
            onesr = cp.tile([1, 128], f32, name="onesr")
            onesrR = cp.tile([1, 128], f32r, name="onesrR")
            nc.gpsimd.dma_start(cos_sb[:], cosg.ap())
            nc.gpsimd.dma_start(sin_sb[:], sing.ap())
            nc.gpsimd.dma_start(mneg_sb[:], mneg.ap())
            make_identity(nc, ident[:])
            nc.vector.tensor_copy(identR[:], ident[:])
            nc.vector.memset(onesc[:], 1.0)
            nc.vector.tensor_copy(onescR[:], onesc[:])
            nc.vector.memset(onesr[:], 1.0)
            nc.vector.tensor_copy(onesrR[:], onesr[:])

            ibs = {(b, h): dp.tile([NCORES, 128, SCW], f32r, name=f"ib{b}{h}")
                   for b in range(B) for h in range(HPC)}
            obs = {(b, h): dp.tile([NCORES, 128, SCW], f32r, name=f"ob{b}{h}")
                   for b in range(B) for h in range(HPC)}

            xc0 = xp.tile([128, KSUB, SC], f32r, tag="xc", name="xc")
            nc.sync.dma_start(xc0[:], xT.ap()[:, :, 0:SC])

            wqk_t = []
            for m in range(2 * HPC):
                wt = wp.tile([128, KSUB, 128], f32r, tag=f"w{m}", name=f"w{m}")
                eng = nc.sync if m < 2 else nc.gpsimd
                eng.dma_start(wt[:],
                              wqkv.ap()[:, :, m * 128:(m + 1) * 128])
                wqk_t.append(wt)
            wv_t = wp.tile([128, KSUB, HPC * HD], f32r, tag="wv", name="wv")
            nc.gpsimd.dma_start(wv_t[:], wqkv.ap()[:, :, VOFF:VOFF + HPC * HD])

            def qkv_rope(b, pre_xc=None):
                qkT = qp.tile([128, 2 * HPC, S], f32r, tag="qkT")
                Vn = qp.tile([128, NKT, HPC * HD], f32r, tag="Vn")
                for sc in range(NQC):
                    if sc == 0 and pre_xc is not None:
                        xc = pre_xc
                    else:
                        xc = xp.tile([128, KSUB, SC], f32r, tag="xc", name="xc")
                        off = b * S + sc * SC
                        nc.sync.dma_start(xc[:], xT.ap()[:, :, off:off + SC])
                    for m in range(2 * HPC):
                        ps = psA.tile([128, 512], f32, tag="bank")
                        for k in range(KSUB):
                            nc.tensor.matmul(
                                ps[:, :SC],
                                wqk_t[m][:, k],
                                xc[:, k],
                                start=(k == 0), stop=(k == KSUB - 1))
                        nc.vector.tensor_copy(
                            qkT[:, m, sc * SC:(sc + 1) * SC], ps[:, :SC])
                    for st2 in range(SC // 128):
                        ps = psA.tile([128, 512], f32, tag="bank")
                        for k in range(KSUB):
                            nc.tensor.matmul(
                                ps[:, :HPC * HD],
                                xc[:, k, st2 * 128:(st2 + 1) * 128],
                                wv_t[:, k],
                                start=(k == 0), stop=(k == KSUB - 1))
                        nc.vector.tensor_copy(
                            Vn[:, sc * (SC // 128) + st2], ps[:, :HPC * HD])

                # RoPE, fused halves (sin grid stored pre-swapped):
                # rt[0:64] = t[64:128]*(-sin); rt[64:128] = t[0:64]*(+sin);
                # t *= cos; t += rt
                for m in range(2 * HPC):
                    rt = rp.tile([128, S], f32, tag="rot", name="rt")
                    nc.vector.tensor_mul(rt[0:64, :],
                                         qkT[64:128, m].bitcast(f32),
                                         sin_sb[64:128, :])
                    nc.vector.tensor_mul(rt[64:128, :],
                                         qkT[0:64, m].bitcast(f32),
                                         sin_sb[0:64, :])
                    nc.vector.tensor_mul(qkT[:, m], qkT[:, m], cos_sb[:])
                    nc.vector.tensor_add(qkT[:, m], qkT[:, m], rt[:])
                return qkT, Vn

            def attention(b, h, qkT, Vn, fillers=()):
                fillers = list(fillers)
                outT = psO.tile([128, S], f32, tag="outT")
                acc = ap_.tile([128, S], f32r, tag="acc")

                def emit_av(kt, off, ets):
                    q0 = 512 * (kt // 4)
                    for c in range(len(ets)):
                        qs = q0 + c * 512
                        o = off if c == 0 else 0
                        nc.tensor.matmul(
                            outT[:, qs + o:qs + 512],
                            Vn[:, kt, h * 128:(h + 1) * 128],
                            ets[c][:, o:512],
                            start=(kt == 0),
                            stop=(kt == 4 * (qs // 512) + 3))

                st = rp.tile([128, S], f32r, tag="rot", name="st")

                def finalize_chunk(j):
                    # denominator: partition-reduce via ones-matmul,
                    # reciprocal, K=1 broadcast matmul, normalize, ship.
                    rps = psA.tile([128, 512], f32, tag="bank")
                    nc.tensor.matmul(rps[0:1, :], onescR[:],
                                     acc[:, j * 512:(j + 1) * 512],
                                     start=True, stop=True)
                    srow = stp.tile([1, 512], f32, tag="srow")
                    nc.vector.reciprocal_approx_fast(srow[:], rps[0:1, :])
                    bp = psA.tile([128, 512], f32, tag="bank")
                    nc.tensor.matmul(bp[:], onesr[:], srow[:],
                                     start=True, stop=True)
                    sl = slice(j * 512, (j + 1) * 512)
                    nc.vector.tensor_copy(st[:, sl], outT[:, sl])
                    nc.vector.tensor_mul(st[:, sl], st[:, sl], bp[:])
                    for jj in (2 * j, 2 * j + 1):
                        nc.sync.dma_start(ibs[(b, h)][jj],
                                          st[:, jj * SCW:(jj + 1) * SCW])

                prev = None
                for kt in range(NKT):
                    q0 = 512 * (kt // 4)
                    off = 128 * (kt % 4)   # causal start within chunk 0
                    nch = (S - q0) // 512
                    sps = []
                    for c in range(nch):
                        sp = psA.tile([128, 512], f32, tag="bank")
                        sps.append(sp)
                        if c == 0:
                            # -1e9 upper-tri mask for the diagonal 128 block
                            nc.tensor.matmul(sp[:, off:512], identR[:],
                                             mneg_sb[:, 0:512 - off],
                                             start=True, stop=False)
                    for c in range(nch):
                        qs = q0 + c * 512
                        o = off if c == 0 else 0
                        nc.tensor.matmul(
                            sps[c][:, o:512],
                            qkT[:, HPC + h, kt * 128:(kt + 1) * 128],
                            qkT[:, h, qs + o:qs + 512],
                            start=(c != 0), stop=True)
                    if prev is not None:
                        emit_av(*prev)
                    if kt >= 6 and (kt - 6) % 4 == 0:
                        finalize_chunk((kt - 6) // 4)
                    ets = []
                    for c in range(nch):
                        o = off if c == 0 else 0
                        et = ep.tile([128, 512], f32r, tag="expT")
                        ets.append(et)
                        nc.scalar.activation(et[:, o:512], sps[c][:, o:512],
                                             EXP, scale=SCALE)
                    for c in range(nch):
                        qs = q0 + c * 512
                        o = off if c == 0 else 0
                        if kt == 0:
                            nc.vector.tensor_copy(acc[:, qs:qs + 512], ets[c][:])
                        else:
                            eng = nc.gpsimd if qs // 512 < 3 else nc.vector
                            eng.tensor_add(acc[:, qs + o:qs + 512],
                                           acc[:, qs + o:qs + 512],
                                           ets[c][:, o:512])
                    if fillers and kt >= 7:
                        fillers.pop(0)()
                    prev = (kt, off, ets)
                emit_av(*prev)
                finalize_chunk(3)
                while fillers:
                    fillers.pop(0)()

            def load_lhs(b, pool, tag):
                # k-subtile order hh*8+i <-> global head 2i+hh (wout is
                # permuted host-side to match)
                lhs = pool.tile([128, KSUB, SCW], f32r, tag=tag,
                                name=f"lhs{b}")
                for hh in range(HPC):
                    nc.sync.dma_start(
                        lhs[:, hh * NCORES:(hh + 1) * NCORES, :],
                        obs[(b, hh)][:].rearrange("i p s -> p i s"))
                return lhs

            def outproj_groups(b, lhs, wos):
                """Closures emitting one (n, m) outproj matmul group each;
                wo chunks prefetched one n ahead (xc slots, shareable)."""

                def fetch(n):
                    if n < 4 and n not in wos:
                        wo = xp.tile([128, KSUB, 512], f32r, tag="xc",
                                     name="wo")
                        nc.sync.dma_start(
                            wo[:], wout.ap()[:, :, n * 512:(n + 1) * 512])
                        wos[n] = wo

                def make(n, m):
                    def emit():
                        fetch(n)
                        fetch(n + 1)
                        wo = wos[n]
                        ps = psA.tile([128, 512], f32, tag="bank")
                        for k in range(KSUB):
                            nc.tensor.matmul(
                                ps[:],
                                lhs[:, k, m * 128:(m + 1) * 128],
                                wo[:, k],
                                start=(k == 0), stop=(k == KSUB - 1))
                        ys = ep.tile([128, 512], f32, tag="expT", name="ys")
                        nc.vector.tensor_copy(ys[:], ps[:])
                        nc.sync.dma_start(
                            y.ap()[b, m * 128:(m + 1) * 128,
                                   n * 512:(n + 1) * 512],
                            ys[:])
                    return emit

                return [make(n, m) for n in range(4)
                        for m in range(SCW // 128)]

            def a2a(b, h):
                nc.gpsimd.collective_compute(
                    "AllToAll", mybir.AluOpType.bypass,
                    replica_groups=[list(range(NCORES))],
                    ins=[ibs[(b, h)].opt()], outs=[obs[(b, h)].opt()])

            # batch 0 compute; its A2A runs while batch 1 computes;
            # outproj(0) slots into PE after batch 1's first head.
            qkT, Vn = qkv_rope(0, pre_xc=xc0)
            attention(0, 0, qkT, Vn)
            a2a(0, 0)
            attention(0, 1, qkT, Vn)
            a2a(0, 1)
            qkT, Vn = qkv_rope(1)
            attention(1, 0, qkT, Vn)
            a2a(1, 0)
            wos = {}

            def prefetch_wo(n):
                wo = xp.tile([128, KSUB, 512], f32r, tag="xc", name="wo")
                nc.sync.dma_start(wo[:],
                                  wout.ap()[:, :, n * 512:(n + 1) * 512])
                wos[n] = wo

            prefetch_wo(0)
            prefetch_wo(1)
            attention(1, 1, qkT, Vn)
            a2a(1, 1)
            lhs0 = load_lhs(0, wp, "wv")
            lhs1 = load_lhs(1, qp, "Vn")
            g0 = outproj_groups(0, lhs0, wos)
            g1 = outproj_groups(1, lhs1, wos)
            for n in range(4):
                g0[2 * n](); g0[2 * n + 1]()
                g1[2 * n](); g1[2 * n + 1]()

    nc.finalize()
    return nc


def _host_inputs(x, w_qkv, w_out):
    xTr = np.ascontiguousarray(
        x.reshape(B * S, D).T.reshape(KSUB, 128, B * S).transpose(1, 0, 2))
    horder = [2 * i + hh for hh in range(HPC) for i in range(NCORES)]
    woutr = np.ascontiguousarray(
        w_out.reshape(H, HD, D)[horder].transpose(1, 0, 2))

    half = HD // 2
    inv = (1.0 / (ROPE_BASE ** (np.arange(half, dtype=np.float32) / half))
           ).astype(np.float32)
    ang = (np.arange(S, dtype=np.float32)[:, None] * inv[None, :])  # [S, 64]
    c = np.cos(ang).astype(np.float32).T      # [64, S]
    s = np.sin(ang).astype(np.float32).T
    cosg = np.ascontiguousarray(np.concatenate([c, c], axis=0))
    # pre-swapped: rows 0:64 = +sin (consumed against t[0:64] -> rt[64:128]),
    # rows 64:128 = -sin (consumed against t[64:128] -> rt[0:64])
    sing = np.ascontiguousarray(np.concatenate([s, -s], axis=0))

    # mneg[p, j] = 0 where j >= p else -1e9 (upper-tri of the diagonal
    # 128-block, padded to 512 query columns).
    u = np.arange(512)[None, :]
    p = np.arange(128)[:, None]
    mneg = np.where(u >= p, 0.0, -1e9).astype(np.float32)

    maps = []
    for i in range(NCORES):
        h0, h1 = 2 * i, 2 * i + 1
        blocks = []
        for base in (0, D, 2 * D):
            blocks.append(w_qkv[:, base + 128 * h0:base + 128 * (h0 + 1)])
            blocks.append(w_qkv[:, base + 128 * h1:base + 128 * (h1 + 1)])
        shard = np.concatenate(blocks, axis=1)  # [D, 768]
        shard = np.ascontiguousarray(
            shard.reshape(KSUB, 128, 3 * HPC * HD).transpose(1, 0, 2))
        maps.append({"xT": xTr, "wqkv": shard, "wout": woutr,
                     "cosg": cosg, "sing": sing, "mneg": mneg})
    return maps


def kernel(x, w_qkv, w_out):
    from concourse.bass_utils import run_bass_kernel_spmd

    x = np.asarray(x, dtype=np.float32)
    w_qkv = np.asarray(w_qkv, dtype=np.float32)
    w_out = np.asarray(w_out, dtype=np.float32)

    if "nc" not in _CACHE:
        _CACHE["nc"] = _build()
    nc = _CACHE["nc"]

    trace = bool(int(os.environ.get("KERNEL_TRACE", "0")))
    if trace:
        trace = _install_trace_shim()

    in_maps = _host_inputs(x, w_qkv, w_out)
    res = run_bass_kernel_spmd(nc, in_maps, core_ids=list(range(NCORES)),
                               trace=trace)
    _CACHE["last_result"] = res
    # y per core i: [B, 256, D] = output rows [b*2048 + i*256, +256)
    full = np.empty((B * S, D), dtype=np.float32)
    for i in range(NCORES):
        yi = res.results[i]["y"]
        for b in range(B):
            full[b * S + i * SCW: b * S + (i + 1) * SCW] = yi[b]
    return full.reshape(B, S, D)



# revision 10
# speedup vs baseline: 1.0680x; 1.0680x over previous
"""Trainium2 Bass kernel for causal multi-head attention with RoPE.

Problem: x[2,2048,2048] -> qkv proj -> RoPE(q,k) -> causal softmax attention
(16 heads, hd=128) -> out proj.  Sharding: tensor-parallel over heads
(2 heads/core x 8 cores); the output projection contraction is restored
with one AllToAll per (batch, head) (head-shards -> sequence-shards), so
each core computes a disjoint [2, 256, 2048] slice of the final output.

v2: all matmul operands are bf16 (PSUM accumulation stays fp32), which
makes LDWEIGHTS (107ns) hide completely under N=512 matmuls and halves
the AllToAll payload.  The causal mask is applied post-exp with a DVE
affine_select on the diagonal 128-block (no PE mask matmuls).  Softmax
denominators: PE ones-matmul partition-reduce -> DVE reciprocal ->
gpsimd partition_broadcast (no PE broadcast matmul).  RoPE is applied
per 512-chunk right after each projection copy so attention starts
almost immediately after the last chunk.  Batch-0's output projection
runs as filler groups inside batch-1's second attention head; all of
w_out is prefetched during attention so the tail is only the last
AllToAll + batch-1's projection.
"""

import os
import sys

if "/opt/trn_rl_repo" not in sys.path:
    sys.path.insert(0, "/opt/trn_rl_repo")

import numpy as np
import ml_dtypes

BF16 = ml_dtypes.bfloat16

B, S, D = 2, 2048, 2048
H, HD = 16, 128
NCORES = 8
HPC = H // NCORES          # heads per core (2)
ROPE_BASE = 10000.0
SCALE = 1.0 / float(np.sqrt(HD))
SC = 512                   # QKV matmul free-dim chunk (s positions)
KSUB = D // 128            # 16 contraction subtiles
SCW = S // NCORES          # 256: per-core output cols per batch
NQC = S // SC              # 4 qkv s-chunks per batch
NKT = S // 128             # 16 key tiles
VOFF = 2 * HPC * HD        # v block column offset in w shard (512)
FILL_KTS = (4, 5, 7, 8, 9, 11, 12, 13, 15)   # filler slots (avoid finalize kts)

_CACHE = {}


def _install_trace_shim():
    """Optionally register the axon NTFF profile hook (for test.py tracing)."""
    try:
        import types

        if "antenv.axon_hooks" in sys.modules:
            return True
        import antenv
        from trn_agent_boot.trn_boot import _ntff_profile_via_ctypes

        hook = _ntff_profile_via_ctypes("/opt/axon/libaxon_pjrt.so")
        mod = types.ModuleType("antenv.axon_hooks")
        _state = {"hook": hook}
        mod.get_axon_ntff_profile_hook = lambda: _state["hook"]
        mod.set_axon_ntff_profile_hook = lambda h: _state.__setitem__("hook", h)
        sys.modules["antenv.axon_hooks"] = mod
        antenv.axon_hooks = mod
        return True
    except Exception:
        return False


def _build():
    import concourse.bass as bass  # noqa: F401
    import concourse.mybir as mybir
    import concourse.tile as tile
    from concourse import bacc

    f32 = mybir.dt.float32
    f32r = mybir.dt.float32r
    bf16 = mybir.dt.bfloat16
    EXP = mybir.ActivationFunctionType.Exp

    nc = bacc.Bacc("TRN2", target_bir_lowering=False, debug=False,
                   num_devices=NCORES)

    xT = nc.dram_tensor("xT", [128, KSUB, B * S], bf16, kind="ExternalInput")
    wqkv = nc.dram_tensor("wqkv", [128, KSUB, 3 * HPC * HD], bf16,
                          kind="ExternalInput")
    wout = nc.dram_tensor("wout", [128, KSUB, D], bf16, kind="ExternalInput")
    cosg = nc.dram_tensor("cosg", [128, S], bf16, kind="ExternalInput")
    sing = nc.dram_tensor("sing", [128, S], bf16, kind="ExternalInput")
    y = nc.dram_tensor("y", [B, SCW, D], f32, kind="ExternalOutput")

    with tile.TileContext(nc) as tc:
        with tc.tile_pool(name="const", bufs=1) as cp, \
             tc.tile_pool(name="dram", bufs=1, space="DRAM") as dp, \
             tc.tile_pool(name="psO", bufs=1, space="PSUM") as psO, \
             tc.tile_pool(name="psA", bufs=2, space="PSUM") as psA, \
             tc.tile_pool(name="psD", bufs=1, space="PSUM") as psD, \
             tc.tile_pool(name="psF", bufs=1, space="PSUM") as psF, \
             tc.tile_pool(name="w", bufs=1) as wp, \
             tc.tile_pool(name="xc", bufs=2) as xp, \
             tc.tile_pool(name="qkv", bufs=1) as qp, \
             tc.tile_pool(name="attn", bufs=1) as ap_, \
             tc.tile_pool(name="stp", bufs=2) as stp, \
             tc.tile_pool(name="rot", bufs=2) as rp, \
             tc.tile_pool(name="exp", bufs=8) as ep, \
             tc.tile_pool(name="row", bufs=2) as sp_, \
             tc.tile_pool(name="den", bufs=2) as dn, \
             tc.tile_pool(name="lhs", bufs=2) as lp, \
             tc.tile_pool(name="wo", bufs=4) as wop, \
             tc.tile_pool(name="ys", bufs=2) as yp:

            cos_sb = cp.tile([128, S], bf16, name="cos_sb")
            sin_sb = cp.tile([128, S], bf16, name="sin_sb")
            onesc = cp.tile([128, 1], f32, name="onesc")
            onescR = cp.tile([128, 1], f32r, name="onescR")
            nc.vector.memset(onesc[:], 1.0)
            nc.vector.tensor_copy(onescR[:], onesc[:])

            # batch-0 chunk-0 x and the first weight group load in parallel
            # on separate queues so the first matmul starts ~6us in.
            xc0 = xp.tile([128, KSUB, SC], bf16, tag="xc", name="xc")
            nc.sync.dma_start(xc0[:], xT.ap()[:, :, 0:SC])
            wqk_t = []
            for m in range(2 * HPC):
                wt = wp.tile([128, KSUB, 128], bf16, tag=f"w{m}", name=f"w{m}")
                eng = nc.gpsimd if m % 2 == 0 else nc.scalar
                eng.dma_start(wt[:], wqkv.ap()[:, :, m * 128:(m + 1) * 128])
                wqk_t.append(wt)
            wv_t = wp.tile([128, KSUB, HPC * HD], bf16, tag="wv", name="wv")
            nc.gpsimd.dma_start(wv_t[:], wqkv.ap()[:, :, VOFF:VOFF + HPC * HD])
            nc.scalar.dma_start(cos_sb[:], cosg.ap())
            nc.scalar.dma_start(sin_sb[:], sing.ap())

            ibs = {(b, h): dp.tile([NCORES, 128, SCW], bf16, name=f"ib{b}{h}")
                   for b in range(B) for h in range(HPC)}
            obs = {(b, h): dp.tile([NCORES, 128, SCW], bf16, name=f"ob{b}{h}")
                   for b in range(B) for h in range(HPC)}

            def qkv_rope(b, pre_xc):
                qkT = qp.tile([128, 2 * HPC, S], bf16, tag="qkT")
                Vn = qp.tile([128, NKT, HPC * HD], bf16, tag="Vn")
                xcs = [pre_xc]
                for sc in range(NQC):
                    xc = xcs[sc]
                    if sc + 1 < NQC:
                        # prefetch next chunk (scalar queue, one ahead)
                        nxt = xp.tile([128, KSUB, SC], bf16, tag="xc",
                                      name="xc")
                        off = b * S + (sc + 1) * SC
                        nc.scalar.dma_start(nxt[:], xT.ap()[:, :, off:off + SC])
                        xcs.append(nxt)
                    sl = slice(sc * SC, (sc + 1) * SC)
                    for m in (0, 2, 1, 3):   # q0, k0, q1, k1
                        ps = psA.tile([128, 512], f32, tag="bank")
                        for k in range(KSUB):
                            nc.tensor.matmul(
                                ps[:, :SC],
                                wqk_t[m][:, k],
                                xc[:, k],
                                start=(k == 0), stop=(k == KSUB - 1))
                        nc.scalar.copy(qkT[:, m, sl], ps[:, :SC])
                        # RoPE, fused halves (sin grid stored pre-swapped):
                        # rt[0:64] = t[64:128]*(-sin); rt[64:128] = t[0:64]*sin
                        # t *= cos; t += rt
                        rt = rp.tile([128, SC], bf16, tag="rot", name="rt")
                        nc.vector.tensor_mul(rt[0:64, :],
                                             qkT[64:128, m, sl],
                                             sin_sb[64:128, sl])
                        nc.vector.tensor_mul(rt[64:128, :],
                                             qkT[0:64, m, sl],
                                             sin_sb[0:64, sl])
                        nc.vector.tensor_mul(qkT[:, m, sl], qkT[:, m, sl],
                                             cos_sb[:, sl])
                        nc.vector.tensor_add(qkT[:, m, sl], qkT[:, m, sl],
                                             rt[:])
                    for st2 in range(SC // 128):
                        ps = psA.tile([128, 512], f32, tag="bank")
                        for k in range(KSUB):
                            nc.tensor.matmul(
                                ps[:, :HPC * HD],
                                xc[:, k, st2 * 128:(st2 + 1) * 128],
                                wv_t[:, k],
                                start=(k == 0), stop=(k == KSUB - 1))
                        nc.scalar.copy(Vn[:, sc * (SC // 128) + st2],
                                       ps[:, :HPC * HD])
                return qkT, Vn

            def attention(b, h, qkT, Vn, fillers=()):
                fillers = list(fillers)
                outT = psO.tile([128, S], f32, tag="outT")
                acc = ap_.tile([128, S], f32r, tag="acc")
                st = stp.tile([128, S], bf16, tag="st")

                def emit_av(kt, off, ets):
                    q0 = 512 * (kt // 4)
                    for c in range(len(ets)):
                        qs = q0 + c * 512
                        o = off if c == 0 else 0
                        nc.tensor.matmul(
                            outT[:, qs + o:qs + 512],
                            Vn[:, kt, h * 128:(h + 1) * 128],
                            ets[c][:, o:512],
                            start=(kt == 0),
                            stop=(kt == 4 * (qs // 512) + 3))

                def finalize_chunk(j):
                    # denom: partition-reduce ones-matmul, fp32 reciprocal,
                    # gpsimd partition-broadcast, normalize, ship to DRAM.
                    sl = slice(j * 512, (j + 1) * 512)
                    rps = psD.tile([128, 512], f32, tag="dps", name="rps")
                    nc.tensor.matmul(rps[0:1, :], onescR[:], acc[:, sl],
                                     start=True, stop=True)
                    srow = sp_.tile([1, 512], f32, tag="srow")
                    nc.vector.reciprocal_approx_fast(srow[:], rps[0:1, :])
                    srb = sp_.tile([1, 512], bf16, tag="srb")
                    nc.vector.tensor_copy(srb[:], srow[:])
                    den = dn.tile([128, 512], bf16, tag="den")
                    nc.gpsimd.partition_broadcast(den[:], srb[:])
                    nc.vector.tensor_copy(st[:, sl], outT[:, sl])
                    nc.vector.tensor_mul(st[:, sl], st[:, sl], den[:])
                    for jj in (2 * j, 2 * j + 1):
                        nc.gpsimd.dma_start(ibs[(b, h)][jj],
                                            st[:, jj * SCW:(jj + 1) * SCW])

                def emit_score(kt, c, off):
                    q0 = 512 * (kt // 4)
                    qs = q0 + c * 512
                    o = off if c == 0 else 0
                    sp = psA.tile([128, 512], f32, tag="bank")
                    nc.tensor.matmul(
                        sp[:, o:512],
                        qkT[:, HPC + h, kt * 128:(kt + 1) * 128],
                        qkT[:, h, qs + o:qs + 512],
                        start=True, stop=True)
                    return sp

                prev = None
                for kt in range(NKT):
                    q0 = 512 * (kt // 4)
                    off = 128 * (kt % 4)
                    nch = (S - q0) // 512
                    sps = [emit_score(kt, c, off)
                           for c in range(min(nch, 2))]
                    if prev is not None:
                        emit_av(*prev)
                    sps += [emit_score(kt, c, off)
                            for c in range(2, nch)]
                    if kt >= 6 and (kt - 6) % 4 == 0:
                        finalize_chunk((kt - 6) // 4)
                    ets = []
                    for c in range(nch):
                        o = off if c == 0 else 0
                        et = ep.tile([128, 512], bf16, tag="expT")
                        ets.append(et)
                        nc.scalar.activation(et[:, o:512], sps[c][:, o:512],
                                             EXP, scale=SCALE)
                    # causal mask of the diagonal 128-block: zero where q < k
                    nc.gpsimd.affine_select(
                        out=ets[0][:, off:off + 128],
                        in_=ets[0][:, off:off + 128],
                        compare_op=mybir.AluOpType.is_ge,
                        fill=0.0, base=0,
                        pattern=[[1, 128]], channel_multiplier=-1)
                    for c in range(nch):
                        qs = q0 + c * 512
                        o = off if c == 0 else 0
                        if kt == 0:
                            nc.vector.tensor_copy(acc[:, qs:qs + 512],
                                                  ets[c][:])
                        else:
                            eng = nc.gpsimd if qs // 512 < 2 else nc.vector
                            eng.tensor_add(acc[:, qs + o:qs + 512],
                                           acc[:, qs + o:qs + 512],
                                           ets[c][:, o:512])
                    if fillers and kt in FILL_KTS:
                        fillers.pop(0)()
                    prev = (kt, off, ets)
                emit_av(*prev)
                finalize_chunk(3)
                while fillers:
                    fillers.pop(0)()

            def a2a(b, h):
                nc.gpsimd.collective_compute(
                    "AllToAll", mybir.AluOpType.bypass,
                    replica_groups=[list(range(NCORES))],
                    ins=[ibs[(b, h)].opt()], outs=[obs[(b, h)].opt()])

            def load_lhs_part(b, hh, lhs):
                # k-subtile order hh*8+i <-> global head 2i+hh (wout is
                # permuted host-side to match)
                nc.sync.dma_start(
                    lhs[:, hh * NCORES:(hh + 1) * NCORES, :],
                    obs[(b, hh)][:].rearrange("i p s -> p i s"))

            wos = {}

            def op_group(b, n, m, lhs):
                def emit():
                    wo = wos[n]
                    ps = psF.tile([128, 512], f32, tag="fbank")
                    for k in range(KSUB):
                        nc.tensor.matmul(
                            ps[:],
                            lhs[:, k, m * 128:(m + 1) * 128],
                            wo[:, k],
                            start=(k == 0), stop=(k == KSUB - 1))
                    ys = yp.tile([128, 512], f32, tag="ys", name="ys")
                    nc.scalar.copy(ys[:], ps[:])
                    nc.scalar.dma_start(
                        y.ap()[b, m * 128:(m + 1) * 128,
                               n * 512:(n + 1) * 512],
                        ys[:])
                return emit

            # ---- schedule ----
            qkT0, Vn0 = qkv_rope(0, xc0)
            attention(0, 0, qkT0, Vn0)
            a2a(0, 0)
            lhs0 = lp.tile([128, KSUB, SCW], bf16, tag="lhs", name="lhs0")
            load_lhs_part(0, 0, lhs0)
            # pre-issue batch-1 chunk-0 x load (runs during attention(0,1))
            xc10 = xp.tile([128, KSUB, SC], bf16, tag="xc", name="xc")
            nc.gpsimd.dma_start(xc10[:], xT.ap()[:, :, S:S + SC])
            attention(0, 1, qkT0, Vn0)
            a2a(0, 1)
            load_lhs_part(0, 1, lhs0)
            qkT1, Vn1 = qkv_rope(1, xc10)
            attention(1, 0, qkT1, Vn1)
            a2a(1, 0)
            lhs1 = lp.tile([128, KSUB, SCW], bf16, tag="lhs", name="lhs1")
            load_lhs_part(1, 0, lhs1)
            # prefetch all of w_out while attention(1,1) runs
            for n in range(4):
                wo = wop.tile([128, KSUB, 512], bf16, tag="wo",
                              name=f"wo{n}")
                nc.gpsimd.dma_start(wo[:],
                                    wout.ap()[:, :, n * 512:(n + 1) * 512])
                wos[n] = wo
            fillers = [op_group(0, n, m, lhs0)
                       for n in range(4) for m in range(SCW // 128)]
            attention(1, 1, qkT1, Vn1, fillers)
            a2a(1, 1)
            load_lhs_part(1, 1, lhs1)
            for n in range(4):
                for m in range(SCW // 128):
                    op_group(1, n, m, lhs1)()

    nc.finalize()
    return nc


def _host_inputs(x, w_qkv, w_out):
    xTr = np.ascontiguousarray(
        x.reshape(B * S, D).T.reshape(KSUB, 128, B * S).transpose(1, 0, 2)
    ).astype(BF16)
    horder = [2 * i + hh for hh in range(HPC) for i in range(NCORES)]
    woutr = np.ascontiguousarray(
        w_out.reshape(H, HD, D)[horder].transpose(1, 0, 2)).astype(BF16)

    half = HD // 2
    inv = (1.0 / (ROPE_BASE ** (np.arange(half, dtype=np.float32) / half))
           ).astype(np.float32)
    ang = (np.arange(S, dtype=np.float32)[:, None] * inv[None, :])  # [S, 64]
    c = np.cos(ang).astype(np.float32).T      # [64, S]
    s = np.sin(ang).astype(np.float32).T
    cosg = np.ascontiguousarray(np.concatenate([c, c], axis=0)).astype(BF16)
    # pre-swapped: rows 0:64 = +sin (consumed against t[0:64] -> rt[64:128]),
    # rows 64:128 = -sin (consumed against t[64:128] -> rt[0:64])
    sing = np.ascontiguousarray(np.concatenate([s, -s], axis=0)).astype(BF16)

    maps = []
    for i in range(NCORES):
        h0, h1 = 2 * i, 2 * i + 1
        blocks = []
        for base in (0, D, 2 * D):
            blocks.append(w_qkv[:, base + 128 * h0:base + 128 * (h0 + 1)])
            blocks.append(w_qkv[:, base + 128 * h1:base + 128 * (h1 + 1)])
        shard = np.concatenate(blocks, axis=1)  # [D, 768]
        shard = np.ascontiguousarray(
            shard.reshape(KSUB, 128, 3 * HPC * HD).transpose(1, 0, 2)
        ).astype(BF16)
        maps.append({"xT": xTr, "wqkv": shard, "wout": woutr,
                     "cosg": cosg, "sing": sing})
    return maps


def kernel(x, w_qkv, w_out):
    from concourse.bass_utils import run_bass_kernel_spmd

    x = np.asarray(x, dtype=np.float32)
    w_qkv = np.asarray(w_qkv, dtype=np.float32)
    w_out = np.asarray(w_out, dtype=np.float32)

    if "nc" not in _CACHE:
        _CACHE["nc"] = _build()
    nc = _CACHE["nc"]

    trace = bool(int(os.environ.get("KERNEL_TRACE", "0")))
    if trace:
        trace = _install_trace_shim()

    in_maps = _host_inputs(x, w_qkv, w_out)
    kw = {}
    if trace and bool(int(os.environ.get("KERNEL_TRACE_ALL", "0"))):
        kw = {"trace_cores": list(range(NCORES)), "stitch_traces": True}
    res = run_bass_kernel_spmd(nc, in_maps, core_ids=list(range(NCORES)),
                               trace=trace, **kw)
    _CACHE["last_result"] = res
    # y per core i: [B, 256, D] = output rows [b*2048 + i*256, +256)
    full = np.empty((B * S, D), dtype=np.float32)
    for i in range(NCORES):
        yi = res.results[i]["y"]
        for b in range(B):
            full[b * S + i * SCW: b * S + (i + 1) * SCW] = yi[b]
    return full.reshape(B, S, D)


# revision 21
# speedup vs baseline: 1.3760x; 1.2884x over previous
"""Trainium2 Bass kernel for causal multi-head attention with RoPE.

Problem: x[2,2048,2048] -> qkv proj -> RoPE(q,k) -> causal softmax attention
(16 heads, hd=128) -> out proj.  Sharding: tensor-parallel over heads
(2 heads/core x 8 cores); the output projection contraction is restored
with one AllToAll per (batch, head) (head-shards -> sequence-shards), so
each core computes a disjoint [2, 256, 2048] slice of the final output.

v2: all matmul operands are bf16 (PSUM accumulation stays fp32), which
makes LDWEIGHTS (107ns) hide completely under N=512 matmuls and halves
the AllToAll payload.  The causal mask is applied post-exp with a DVE
affine_select on the diagonal 128-block (no PE mask matmuls).  Softmax
denominators: PE ones-matmul partition-reduce -> DVE reciprocal ->
gpsimd partition_broadcast (no PE broadcast matmul).  RoPE is applied
per 512-chunk right after each projection copy so attention starts
almost immediately after the last chunk.  Batch-0's output projection
runs as filler groups inside batch-1's second attention head; all of
w_out is prefetched during attention so the tail is only the last
AllToAll + batch-1's projection.
"""

import os
import sys

if "/opt/trn_rl_repo" not in sys.path:
    sys.path.insert(0, "/opt/trn_rl_repo")

import numpy as np
import ml_dtypes

BF16 = ml_dtypes.bfloat16

B, S, D = 2, 2048, 2048
H, HD = 16, 128
NCORES = 8
HPC = H // NCORES          # heads per core (2)
ROPE_BASE = 10000.0
SCALE = 1.0 / float(np.sqrt(HD))
SC = 512                   # QKV matmul free-dim chunk (s positions)
KSUB = D // 128            # 16 contraction subtiles
SCW = S // NCORES          # 256: per-core output cols per batch
NQC = S // SC              # 4 qkv s-chunks per batch
NKT = S // 128             # 16 key tiles
VOFF = 2 * HPC * HD        # v block column offset in w shard (512)
FILL_KTS = (4, 5, 7, 8, 9, 11, 12, 13, 15)   # filler slots (avoid finalize kts)

_CACHE = {}


def _install_trace_shim():
    """Optionally register the axon NTFF profile hook (for test.py tracing)."""
    try:
        import types

        if "antenv.axon_hooks" in sys.modules:
            return True
        import antenv
        from trn_agent_boot.trn_boot import _ntff_profile_via_ctypes

        hook = _ntff_profile_via_ctypes("/opt/axon/libaxon_pjrt.so")
        mod = types.ModuleType("antenv.axon_hooks")
        _state = {"hook": hook}
        mod.get_axon_ntff_profile_hook = lambda: _state["hook"]
        mod.set_axon_ntff_profile_hook = lambda h: _state.__setitem__("hook", h)
        sys.modules["antenv.axon_hooks"] = mod
        antenv.axon_hooks = mod
        return True
    except Exception:
        return False


def _build():
    import concourse.bass as bass  # noqa: F401
    import concourse.mybir as mybir
    import concourse.tile as tile
    from concourse import bacc

    f32 = mybir.dt.float32
    f32r = mybir.dt.float32r
    bf16 = mybir.dt.bfloat16
    EXP = mybir.ActivationFunctionType.Exp

    nc = bacc.Bacc("TRN2", target_bir_lowering=False, debug=False,
                   num_devices=NCORES)

    xT = nc.dram_tensor("xT", [128, KSUB, B * S], bf16, kind="ExternalInput")
    wqkv = nc.dram_tensor("wqkv", [128, KSUB, 3 * HPC * HD], bf16,
                          kind="ExternalInput")
    wout = nc.dram_tensor("wout", [128, KSUB, D], bf16, kind="ExternalInput")
    cosg = nc.dram_tensor("cosg", [128, S], bf16, kind="ExternalInput")
    sing = nc.dram_tensor("sing", [128, S], bf16, kind="ExternalInput")
    mneg = nc.dram_tensor("mneg", [128, 512], bf16, kind="ExternalInput")
    y = nc.dram_tensor("y", [B, SCW, D], f32, kind="ExternalOutput")

    with tile.TileContext(nc) as tc:
        with tc.tile_pool(name="const", bufs=1) as cp, \
             tc.tile_pool(name="dram", bufs=1, space="DRAM") as dp, \
             tc.tile_pool(name="psO", bufs=1, space="PSUM") as psO, \
             tc.tile_pool(name="psA", bufs=2, space="PSUM") as psA, \
             tc.tile_pool(name="psD", bufs=1, space="PSUM") as psD, \
             tc.tile_pool(name="psF", bufs=1, space="PSUM") as psF, \
             tc.tile_pool(name="w", bufs=1) as wp, \
             tc.tile_pool(name="xc", bufs=2) as xp, \
             tc.tile_pool(name="qkv", bufs=1) as qp, \
             tc.tile_pool(name="attn", bufs=1) as ap_, \
             tc.tile_pool(name="stp", bufs=2) as stp, \
             tc.tile_pool(name="rot", bufs=2) as rp, \
             tc.tile_pool(name="exp", bufs=8) as ep, \
             tc.tile_pool(name="row", bufs=2) as sp_, \
             tc.tile_pool(name="lhs", bufs=2) as lp, \
             tc.tile_pool(name="wo", bufs=4) as wop, \
             tc.tile_pool(name="ys", bufs=2) as yp:

            from concourse.masks import make_identity

            cos_sb = cp.tile([128, S], bf16, name="cos_sb")
            sin_sb = cp.tile([128, S], bf16, name="sin_sb")
            mneg_sb = cp.tile([128, 512], bf16, name="mneg_sb")
            identF = cp.tile([128, 128], f32, name="identF")
            identB = cp.tile([128, 128], bf16, name="identB")
            onesc = cp.tile([128, 1], f32, name="onesc")
            onescR = cp.tile([128, 1], f32r, name="onescR")
            onesr = cp.tile([1, 128], bf16, name="onesr")
            nc.vector.memset(onesc[:], 1.0)
            nc.vector.tensor_copy(onescR[:], onesc[:])
            nc.vector.memset(onesr[:], 1.0)
            make_identity(nc, identF[:])
            nc.vector.tensor_copy(identB[:], identF[:])

            # batch-0 chunk-0 x and the whole qkv weight shard load in
            # parallel on separate queues so the first matmul starts ~6us in.
            xc0 = xp.tile([128, KSUB, SC], bf16, tag="xc", name="xc")
            nc.sync.dma_start(xc0[:], xT.ap()[:, :, 0:SC])
            wAll = wp.tile([128, KSUB, 3 * HPC * HD], bf16, name="wAll")
            nc.gpsimd.dma_start(wAll[:], wqkv.ap())
            nc.scalar.dma_start(cos_sb[:], cosg.ap())
            nc.scalar.dma_start(sin_sb[:], sing.ap())
            nc.scalar.dma_start(mneg_sb[:], mneg.ap())

            ibs = {(b, h): dp.tile([NCORES, 128, SCW], bf16, name=f"ib{b}{h}")
                   for b in range(B) for h in range(HPC)}
            obs = {(b, h): dp.tile([NCORES, 128, SCW], bf16, name=f"ob{b}{h}")
                   for b in range(B) for h in range(HPC)}

            def qkv_rope(b, pre_xc):
                qkT = qp.tile([128, 2 * HPC, S], bf16, tag="qkT")
                Vn = qp.tile([128, NKT, HPC * HD], bf16, tag="Vn")
                xcs = [pre_xc]
                for sc in range(NQC):
                    xc = xcs[sc]
                    if sc + 1 < NQC:
                        # prefetch next chunk (gpsimd queue, one ahead)
                        nxt = xp.tile([128, KSUB, SC], bf16, tag="xc",
                                      name="xc")
                        off = b * S + (sc + 1) * SC
                        nc.gpsimd.dma_start(nxt[:],
                                            xT.ap()[:, :, off:off + SC])
                        xcs.append(nxt)
                    sl = slice(sc * SC, (sc + 1) * SC)
                    for m in (0, 2, 1, 3):   # q0, k0, q1, k1
                        ps = psA.tile([128, 512], f32, tag="bank")
                        for k in range(KSUB):
                            nc.tensor.matmul(
                                ps[:, :SC],
                                wAll[:, k, m * 128:(m + 1) * 128],
                                xc[:, k],
                                start=(k == 0), stop=(k == KSUB - 1))
                        nc.vector.tensor_copy(qkT[:, m, sl], ps[:, :SC])
                        # RoPE, fused halves (sin grid stored pre-swapped):
                        # rt[0:64] = t[64:128]*(-sin); rt[64:128] = t[0:64]*sin
                        # t *= cos; t += rt
                        rt = rp.tile([128, SC], bf16, tag="rot", name="rt")
                        nc.vector.tensor_mul(rt[0:64, :],
                                             qkT[64:128, m, sl],
                                             sin_sb[64:128, sl])
                        nc.vector.tensor_mul(rt[64:128, :],
                                             qkT[0:64, m, sl],
                                             sin_sb[0:64, sl])
                        nc.vector.tensor_mul(qkT[:, m, sl], qkT[:, m, sl],
                                             cos_sb[:, sl])
                        nc.vector.tensor_add(qkT[:, m, sl], qkT[:, m, sl],
                                             rt[:])
                    for st2 in range(SC // 128):
                        ps = psA.tile([128, 512], f32, tag="bank")
                        for k in range(KSUB):
                            nc.tensor.matmul(
                                ps[:, :HPC * HD],
                                xc[:, k, st2 * 128:(st2 + 1) * 128],
                                wAll[:, k, VOFF:VOFF + HPC * HD],
                                start=(k == 0), stop=(k == KSUB - 1))
                        nc.vector.tensor_copy(Vn[:, sc * (SC // 128) + st2],
                                              ps[:, :HPC * HD])
                return qkT, Vn

            def attention(b, h, qkT, Vn, fillers=()):
                fillers = list(fillers)
                outT = psO.tile([128, S], f32, tag="outT")
                acc = ap_.tile([128, S], f32r, tag="acc")
                st = stp.tile([128, S], bf16, tag="st")

                def emit_av(kt, off, ets):
                    q0 = 512 * (kt // 4)
                    for c in range(len(ets)):
                        qs = q0 + c * 512
                        o = off if c == 0 else 0
                        nc.tensor.matmul(
                            outT[:, qs + o:qs + 512],
                            Vn[:, kt, h * 128:(h + 1) * 128],
                            ets[c][:, o:512],
                            start=(kt == 0),
                            stop=(kt == 4 * (qs // 512) + 3))

                def finalize_chunk(j):
                    # denom: partition-reduce ones-matmul, fp32 reciprocal,
                    # K=1 broadcast matmul, normalize, ship to DRAM.
                    sl = slice(j * 512, (j + 1) * 512)
                    rps = psD.tile([128, 512], f32, tag="dps", name="rps")
                    nc.tensor.matmul(rps[0:1, :], onescR[:], acc[:, sl],
                                     start=True, stop=True)
                    srow = sp_.tile([1, 512], f32, tag="srow")
                    nc.vector.reciprocal_approx_fast(srow[:], rps[0:1, :])
                    srb = sp_.tile([1, 512], bf16, tag="srb")
                    nc.vector.tensor_copy(srb[:], srow[:])
                    bp = psD.tile([128, 512], f32, tag="dps", name="bp")
                    nc.tensor.matmul(bp[:], onesr[:], srb[:],
                                     start=True, stop=True)
                    nc.vector.tensor_copy(st[:, sl], outT[:, sl])
                    nc.vector.tensor_mul(st[:, sl], st[:, sl], bp[:])
                    for jj in (2 * j, 2 * j + 1):
                        nc.gpsimd.dma_start(ibs[(b, h)][jj],
                                            st[:, jj * SCW:(jj + 1) * SCW])

                def emit_score(kt, c, off):
                    q0 = 512 * (kt // 4)
                    qs = q0 + c * 512
                    o = off if c == 0 else 0
                    sp = psA.tile([128, 512], f32, tag="bank")
                    if c == 0:
                        # -1e9 upper-tri mask for the diagonal 128 block
                        nc.tensor.matmul(sp[:, o:512], identB[:],
                                         mneg_sb[:, 0:512 - o],
                                         start=True, stop=False)
                    nc.tensor.matmul(
                        sp[:, o:512],
                        qkT[:, HPC + h, kt * 128:(kt + 1) * 128],
                        qkT[:, h, qs + o:qs + 512],
                        start=(c != 0), stop=True)
                    return sp

                prev = None
                for kt in range(NKT):
                    q0 = 512 * (kt // 4)
                    off = 128 * (kt % 4)
                    nch = (S - q0) // 512
                    sps = [emit_score(kt, c, off)
                           for c in range(min(nch, 2))]
                    if prev is not None:
                        emit_av(*prev)
                    sps += [emit_score(kt, c, off)
                            for c in range(2, nch)]
                    if kt >= 6 and (kt - 6) % 4 == 0:
                        finalize_chunk((kt - 6) // 4)
                    ets = []
                    for c in range(nch):
                        o = off if c == 0 else 0
                        et = ep.tile([128, 512], bf16, tag="expT")
                        ets.append(et)
                        nc.scalar.activation(et[:, o:512], sps[c][:, o:512],
                                             EXP, scale=SCALE)
                    for c in range(nch):
                        qs = q0 + c * 512
                        o = off if c == 0 else 0
                        if kt == 0:
                            nc.vector.tensor_copy(acc[:, qs:qs + 512],
                                                  ets[c][:])
                        else:
                            eng = nc.gpsimd if qs // 512 < 2 else nc.vector
                            eng.tensor_add(acc[:, qs + o:qs + 512],
                                           acc[:, qs + o:qs + 512],
                                           ets[c][:, o:512])
                    if fillers and kt in FILL_KTS:
                        fillers.pop(0)()
                    prev = (kt, off, ets)
                emit_av(*prev)
                finalize_chunk(3)
                while fillers:
                    fillers.pop(0)()

            def a2a(b, h):
                nc.gpsimd.collective_compute(
                    "AllToAll", mybir.AluOpType.bypass,
                    replica_groups=[list(range(NCORES))],
                    ins=[ibs[(b, h)].opt()], outs=[obs[(b, h)].opt()])

            def load_lhs_part(b, hh, lhs):
                # k-subtile order hh*8+i <-> global head 2i+hh (wout is
                # permuted host-side to match)
                nc.sync.dma_start(
                    lhs[:, hh * NCORES:(hh + 1) * NCORES, :],
                    obs[(b, hh)][:].rearrange("i p s -> p i s"))

            wos = {}

            def op_group(b, n, m, lhs, pool=None):
                def emit():
                    wo = wos[n]
                    pl = pool if pool is not None else psF
                    ps = pl.tile([128, 512], f32,
                                 tag="fbank" if pl is psF else "bank")
                    for k in range(KSUB):
                        nc.tensor.matmul(
                            ps[:],
                            lhs[:, k, m * 128:(m + 1) * 128],
                            wo[:, k],
                            start=(k == 0), stop=(k == KSUB - 1))
                    ys = yp.tile([128, 512], f32, tag="ys", name="ys")
                    nc.vector.tensor_copy(ys[:], ps[:])
                    nc.scalar.dma_start(
                        y.ap()[b, m * 128:(m + 1) * 128,
                               n * 512:(n + 1) * 512],
                        ys[:])
                return emit

            # ---- schedule ----
            qkT0, Vn0 = qkv_rope(0, xc0)
            attention(0, 0, qkT0, Vn0)
            a2a(0, 0)
            lhs0 = lp.tile([128, KSUB, SCW], bf16, tag="lhs", name="lhs0")
            load_lhs_part(0, 0, lhs0)
            # pre-issue batch-1 chunk-0 x load (runs during attention(0,1))
            xc10 = xp.tile([128, KSUB, SC], bf16, tag="xc", name="xc")
            nc.gpsimd.dma_start(xc10[:], xT.ap()[:, :, S:S + SC])
            attention(0, 1, qkT0, Vn0)
            a2a(0, 1)
            load_lhs_part(0, 1, lhs0)
            qkT1, Vn1 = qkv_rope(1, xc10)
            # prefetch all of w_out on the sync queue: issued once a2a(0,1)
            # completes (during attention(1,0)), ready before the fillers.
            for n in range(4):
                wo = wop.tile([128, KSUB, 512], bf16, tag="wo",
                              name=f"wo{n}")
                nc.sync.dma_start(wo[:],
                                  wout.ap()[:, :, n * 512:(n + 1) * 512])
                wos[n] = wo
            attention(1, 0, qkT1, Vn1)
            a2a(1, 0)
            lhs1 = lp.tile([128, KSUB, SCW], bf16, tag="lhs", name="lhs1")
            load_lhs_part(1, 0, lhs1)
            fillers = [op_group(0, n, m, lhs0)
                       for n in range(4) for m in range(SCW // 128)]
            attention(1, 1, qkT1, Vn1, fillers)
            a2a(1, 1)
            load_lhs_part(1, 1, lhs1)
            pools = [psF, psA]
            gi = 0
            for n in range(4):
                for m in range(SCW // 128):
                    op_group(1, n, m, lhs1, pools[gi % 2])()
                    gi += 1

    nc.finalize()
    return nc


def _host_inputs(x, w_qkv, w_out):
    xTr = np.ascontiguousarray(
        x.reshape(B * S, D).T.reshape(KSUB, 128, B * S).transpose(1, 0, 2)
    ).astype(BF16)
    horder = [2 * i + hh for hh in range(HPC) for i in range(NCORES)]
    woutr = np.ascontiguousarray(
        w_out.reshape(H, HD, D)[horder].transpose(1, 0, 2)).astype(BF16)

    half = HD // 2
    inv = (1.0 / (ROPE_BASE ** (np.arange(half, dtype=np.float32) / half))
           ).astype(np.float32)
    ang = (np.arange(S, dtype=np.float32)[:, None] * inv[None, :])  # [S, 64]
    c = np.cos(ang).astype(np.float32).T      # [64, S]
    s = np.sin(ang).astype(np.float32).T
    cosg = np.ascontiguousarray(np.concatenate([c, c], axis=0)).astype(BF16)
    # pre-swapped: rows 0:64 = +sin (consumed against t[0:64] -> rt[64:128]),
    # rows 64:128 = -sin (consumed against t[64:128] -> rt[0:64])
    sing = np.ascontiguousarray(np.concatenate([s, -s], axis=0)).astype(BF16)

    # mneg[p, j] = 0 where j >= p else -1e9 (upper-tri of the diagonal
    # 128-block, padded to 512 query columns).
    u = np.arange(512)[None, :]
    p = np.arange(128)[:, None]
    mneg = np.where(u >= p, 0.0, -1e9).astype(BF16)

    maps = []
    for i in range(NCORES):
        h0, h1 = 2 * i, 2 * i + 1
        blocks = []
        for base in (0, D, 2 * D):
            blocks.append(w_qkv[:, base + 128 * h0:base + 128 * (h0 + 1)])
            blocks.append(w_qkv[:, base + 128 * h1:base + 128 * (h1 + 1)])
        shard = np.concatenate(blocks, axis=1)  # [D, 768]
        shard = np.ascontiguousarray(
            shard.reshape(KSUB, 128, 3 * HPC * HD).transpose(1, 0, 2)
        ).astype(BF16)
        maps.append({"xT": xTr, "wqkv": shard, "wout": woutr,
                     "cosg": cosg, "sing": sing, "mneg": mneg})
    return maps


def kernel(x, w_qkv, w_out):
    from concourse.bass_utils import run_bass_kernel_spmd

    x = np.asarray(x, dtype=np.float32)
    w_qkv = np.asarray(w_qkv, dtype=np.float32)
    w_out = np.asarray(w_out, dtype=np.float32)

    if "nc" not in _CACHE:
        _CACHE["nc"] = _build()
    nc = _CACHE["nc"]

    trace = bool(int(os.environ.get("KERNEL_TRACE", "0")))
    if trace:
        trace = _install_trace_shim()

    in_maps = _host_inputs(x, w_qkv, w_out)
    kw = {}
    if trace and bool(int(os.environ.get("KERNEL_TRACE_ALL", "0"))):
        kw = {"trace_cores": list(range(NCORES)), "stitch_traces": True}
    res = run_bass_kernel_spmd(nc, in_maps, core_ids=list(range(NCORES)),
                               trace=trace, **kw)
    _CACHE["last_result"] = res
    # y per core i: [B, 256, D] = output rows [b*2048 + i*256, +256)
    full = np.empty((B * S, D), dtype=np.float32)
    for i in range(NCORES):
        yi = res.results[i]["y"]
        for b in range(B):
            full[b * S + i * SCW: b * S + (i + 1) * SCW] = yi[b]
    return full.reshape(B, S, D)
